# revision 27
# baseline (speedup 1.0000x reference)
"""Kalman filter (state=16, obs=96, T=8192) on 8 Trainium2 NeuronCores.

Math: with isotropic A=alpha*I, Q=q*I, R=r*I, P0=p0*I the whole Riccati
trajectory is diagonal in the fixed orthonormal eigenbasis U of C^T C
(SVD C = Z diag(sig) U^T).  The filter reduces to 16 independent scalar
recurrences z_t = a_t * z_{t-1} + g_t * (Z^T y_t), x_t = U z_t, with
a_t, g_t from a scalar per-mode Riccati recursion (y-independent, host
precomputed in fp64).

Rescaling zt_t := z_t / g_t turns the update into
    zt_t = at_t * zt_{t-1} + w_t,      at_t = a_t * g_{t-1}/g_t,
    w_t  = (Z^T y_t)_mode,
so the device needs one matmul (Z^T y) and one fused multiply-add scan;
the g_t scaling and the U rotation move into the host-side stitch.

Device schedule per core (1024 steps, all fixed-latency stages overlap):
 - y and Z arrive as one bf16 DMA [96, 1040] (SP/HWDGE); the fp32 decay
   tensor av [128,128] arrives on a second queue (Act/HWDGE).
 - 8 matmuls w_b = y_b^T Z -> PSUM [128, 16] slabs (moving dim 16).
 - one DVE stream-transpose (32x32 blocks) flips PSUM w into scan
   orientation: partition q=32P+16h+m holds 4 x 32-step runs of
   recurrence (block 2F+h, mode m).
 - one tensor_tensor_scan [128,128] runs everything (init 0); segment
   and block carries are rank-1 in scan space and stitched on host in
   fp64 (the device's own outputs give every segment's carry-in).
 - output leaves via a prepare_only kv_writeback whose descriptors were
   generated during the input DMA; after the scan only the trigger_dma
   + transfer + completion-sem remain.  Two IR-level adjustments make
   this work under TileContext (see _build_nc): a decoy source tensor
   (address-patched post-compile) keeps the prep off the scan's WAR
   path, and the epilogue's orphaned DMASW lane waits are pointed at
   the semaphore the descriptor actually bumps.
"""

import numpy as np

STATE = 16
OBS = 96
T = 8192
N_CORES = 8
L = T // N_CORES        # 1024 steps per core
NSB = 8                 # sub-blocks per core
SB = L // NSB           # 128 steps per sub-block
SEG = 32                # stream-transpose square -> scan segment length

_COMPILED = {}


def _build_nc():
    import concourse.tile as tile
    from concourse import bacc, mybir

    f32 = mybir.dt.float32
    bf16 = mybir.dt.bfloat16
    i32 = mybir.dt.int32
    nc = bacc.Bacc("TRN2", target_bir_lowering=False, debug=False,
                   num_devices=N_CORES)

    yz_d = nc.dram_tensor("yz", [OBS, L + 16], bf16, kind="ExternalInput")
    av_d = nc.dram_tensor("av", [128, SB], f32, kind="ExternalInput")
    zo_d = nc.dram_tensor("zo", [1, 128, 1, SB], f32, kind="ExternalOutput")

    with tile.TileContext(nc) as tc:
        with (
            tc.tile_pool(name="pool", bufs=1) as pool,
            tc.tile_pool(name="psum", bufs=1, space="PSUM") as psum,
        ):
            # zdummy is a decoy the kv_writeback prep "reads" so Tile's
            # byte-range tracker sees no overlap with the scan's write to
            # zout (a tracked overlap would order the scan after the prep's
            # DMA-completion tick -> deadlock with the trigger).  After
            # compile, zdummy's address is patched to zout's slot, so the
            # generated descriptors read the real data.
            zout = pool.tile([128, SB], f32)
            zdummy = nc.alloc_sbuf_tensor("zout_decoy", [128, SB], f32)
            ctx = pool.tile([128, 1], i32)
            nc.gpsimd.memset(ctx[:], 0)
            dma_sem = nc.alloc_semaphore("zo_dma")
            # Descriptor generation runs here, overlapped with the input DMA;
            # the data read happens at trigger_dma time, ordered after the
            # scan via the signals_writable WAW edge below.
            nc.gpsimd.kv_writeback(
                zo_d[:, :, :, :],
                zdummy[:].rearrange("p (a b f) -> p a b f", a=1, b=1),
                ctx[:],
                prepare_only=True,
                sem=dma_sem,
            )

            yz = pool.tile([OBS, L + 16], bf16)
            nc.sync.dma_start(yz[:], yz_d[:, :])
            av = pool.tile([128, SB], f32)
            nc.scalar.dma_start(av[:], av_d[:, :])

            zmat = yz[:, L:L + 16]  # Z [96, 16] bf16
            w = psum.tile([128, SB], f32)
            for b in range(NSB):
                # w[:, 16b:16b+16] = y_b^T Z  (partition = step-in-block)
                nc.tensor.matmul(
                    w[:, STATE * b:STATE * (b + 1)],
                    yz[:, SB * b:SB * (b + 1)], zmat,
                    start=True, stop=True,
                )
            wt = pool.tile([128, SB], f32)
            nc.vector.transpose(wt[:], w[:])
            nc.vector.tensor_tensor_scan(
                zout[:], av[:], wt[:], 0.0,
                op0=mybir.AluOpType.mult, op1=mybir.AluOpType.add,
            )
            # signals_writable gives the trigger a tracked WAW edge on zout:
            # the DMA fires only after the scan completes.
            nc.gpsimd.trigger_dma(count=None, signals_writable=[zout[:]])

    nc.compile()

    # Point the decoy at the real scan output (slot address known only after
    # tile allocation) so the generated descriptors read the actual data.
    nc.lookup_mloc(zdummy).addr = nc.lookup_mloc(zout.name).addr


    # Tile books the SWDGE prep on a DMASW lane and makes the epilogue wait
    # on the lane semaphore, but nothing ever bumps it for a prepare_only
    # descriptor whose completion sem was baked in via sem= (it bumps zo_dma
    # instead).  Point every DMASW wait at zo_dma so both the cost model and
    # the hardware wait on the semaphore the descriptor actually increments.
    n_fix = 0
    for bb in nc.m.functions[0].blocks:
        for ins in bb.instructions:
            si = ins.sync_info
            if not si:
                continue
            for w in si.on_wait:
                if (getattr(w, "ant_name", "") or "").startswith("DMASW"):
                    w.id = dma_sem.num
                    n_fix += 1
    assert n_fix >= 1, "expected at least one DMASW drain wait to rewrite"
    return nc


def _host_precompute(A, C, Q, R, x_init, P_init):
    """fp64 y-independent precompute: SVD of C + per-mode scalar Riccati,
    then the rescaled decay at_t = a_t * g_{t-1}/g_t (g_{-1} := 1)."""
    A64 = A.astype(np.float64)
    C64 = C.astype(np.float64)
    alpha = A64[0, 0]
    q = Q.astype(np.float64)[0, 0]
    r = R.astype(np.float64)[0, 0]
    p0 = P_init.astype(np.float64)[0, 0]

    Zs, sig, UT = np.linalg.svd(C64, full_matrices=False)
    U = UT.T

    d = np.full(STATE, p0)
    a_seq = np.empty((T, STATE))
    g_seq = np.empty((T, STATE))
    for t in range(T):
        dp = alpha * alpha * d + q
        g = dp * sig / (sig * sig * dp + r)
        oneminus = 1.0 - sig * g
        a_seq[t] = alpha * oneminus
        g_seq[t] = g
        d = oneminus * dp

    g_prev = np.vstack([np.ones((1, STATE)), g_seq[:-1]])
    at_seq = a_seq * g_prev / g_seq

    z_init = U.T @ x_init.astype(np.float64)  # == zt_{-1} with g_{-1}=1
    return Zs, U, at_seq, g_seq, z_init


def _dev_layout(at_core):
    """[1024, 16] time-major -> [128, 128] device scan layout.

    t = 128*(2F+h) + 32P + j, q = 32P + 16h + m, c' = 32F + j:
    dev[q, c'] = at_core[t, m].
    """
    return (at_core.reshape(4, 2, 4, SEG, STATE)   # [F, h, P, j, m]
            .transpose(2, 1, 4, 0, 3)              # [P, h, m, F, j]
            .reshape(128, SB))


def _time_layout(dev):
    """Inverse of _dev_layout: [128, 128] -> [1024, 16]."""
    return (dev.reshape(4, 2, STATE, 4, SEG)       # [P, h, m, F, j]
            .transpose(3, 1, 0, 4, 2)              # [F, h, P, j, m]
            .reshape(L, STATE))


def _isotropic(M, dim):
    c = M[0, 0]
    return bool(np.abs(M - c * np.eye(dim, dtype=M.dtype)).max() <= 1e-30)


def _fallback(y_seq, A, C, Q, R, x_init, P_init):
    """General (non-isotropic) inputs: plain fp32 numpy filter."""
    f = np.float32
    A = A.astype(f); C = C.astype(f); Q = Q.astype(f); R = R.astype(f)
    x = x_init.astype(f); P = P_init.astype(f)
    I = np.eye(STATE, dtype=f)
    out = np.empty((T, STATE), f)
    for t in range(T):
        x_pred = A @ x
        P_pred = A @ P @ A.T + Q
        S = C @ P_pred @ C.T + R
        K = (P_pred @ C.T @ np.linalg.inv(S)).astype(f)
        x = x_pred + K @ (y_seq[t].astype(f) - C @ x_pred)
        P = ((I - K @ C) @ P_pred).astype(f)
        out[t] = x
    return out


def kernel(y_seq, A, C, Q, R, x_init, P_init):
    import ml_dtypes

    y_seq = np.asarray(y_seq)
    A = np.asarray(A); C = np.asarray(C); Q = np.asarray(Q)
    R = np.asarray(R)
    x_init = np.asarray(x_init); P_init = np.asarray(P_init)

    if not (_isotropic(A, STATE) and _isotropic(Q, STATE)
            and _isotropic(R, OBS) and _isotropic(P_init, STATE)):
        return _fallback(y_seq, A, C, Q, R, x_init, P_init)

    Zs, U, at_seq, g_seq, z_init = _host_precompute(
        A, C, Q, R, x_init, P_init)

    bf = ml_dtypes.bfloat16
    Zb = Zs.astype(bf)

    if "nc" not in _COMPILED:
        _COMPILED["nc"] = _build_nc()
    nc = _COMPILED["nc"]

    in_maps = []
    at_dev64 = []
    for c in range(N_CORES):
        sl = slice(c * L, (c + 1) * L)
        yz = np.empty((OBS, L + 16), bf)
        yz[:, :L] = y_seq[sl].astype(bf).T
        yz[:, L:] = Zb
        ad = _dev_layout(at_seq[sl])                 # fp64 [128, 128]
        at_dev64.append(ad)
        in_maps.append({"yz": yz,
                        "av": np.ascontiguousarray(ad, dtype=np.float32)})

    from concourse.bass_utils import run_bass_kernel_spmd
    res = run_bass_kernel_spmd(nc, in_maps, core_ids=list(range(N_CORES)))

    # --- host stitch (fp64) -------------------------------------------------
    # Device out dev[q, c'] is a sub-block-local scan whose state ran across
    # the 32-element F-segments of each partition row.  For segment (q, F):
    #   true_seg = dev_seg + pseg * (tin - din),  din = dev[q, 32F-1] (0 at F=0)
    # with pseg the in-segment prefix products of at and tin the recurrence's
    # true incoming state (block carry for P=0, previous segment's end else).
    zt_true = np.empty((T, STATE), np.float64)
    carry = z_init.copy()                            # enters global block 0
    for c in range(N_CORES):
        dev = np.asarray(res.results[c]["zo"], np.float64).reshape(128, SB)
        # av layout uses the float32 the device actually multiplied by
        pseg = (at_dev64[c].astype(np.float32).astype(np.float64)
                .reshape(128, 4, SEG).cumprod(axis=2).reshape(128, SB))
        ztc_dev = np.empty((128, SB), np.float64)    # corrected, device layout
        for lb in range(NSB):                        # local block, time order
            F, h = lb // 2, lb % 2
            cs = slice(SEG * F, SEG * (F + 1))
            tin = carry
            for P in range(4):
                rows = slice(32 * P + 16 * h, 32 * P + 16 * h + STATE)
                din = dev[rows, SEG * F - 1] if F > 0 else 0.0
                seg = dev[rows, cs] + pseg[rows, cs] * (tin - din)[:, None]
                ztc_dev[rows, cs] = seg
                tin = seg[:, -1]
            carry = tin
        zt_true[c * L:(c + 1) * L] = _time_layout(ztc_dev)

    x = (g_seq * zt_true) @ U.T
    return x.astype(np.float32)

